# revision 1
# baseline (speedup 1.0000x reference)
"""MPN-COV pooling + projection kernel for 8 Trainium2 NeuronCores.

Problem: nn_PillTeacher_48661979464182
  feat [64, 256, 14, 14] -> per-sample covariance + 3 Newton-Schulz sqrt
  iterations -> L2-normalize -> project with W_proj [512, 65536] -> BN -> L2.

Sharding:
  - Pooling phase: pure data parallel, 8 samples per core.
  - Projection: k-shard of W_proj (each core holds an 8192-wide slice of the
    contraction dim). AllToAll exchanges the normalized pooled matrices so
    every core gets its k-slice of all 64 samples; partial embeddings are
    summed with ReduceScatter back to the owning core of each sample.

Key tricks:
  - Every matrix in the Newton-Schulz iteration is a polynomial of the
    (symmetric) covariance -> symmetric -> matmul lhsT operands read the
    row-major tiles directly (no transposes on device; feat pre-transposed
    on host).
  - The final L2 normalization is invariant to any positive per-sample
    scale, so 1/M, 1/trace, sqrt(trY) and the 0.5 of the last NS Y-update
    all drop out.
  - fp32r (4x-rate fp32 matmul mode) for all matmuls.
  - BN + bias folded into a host-computed scale/shift.

Workarounds for this walrus build:
  - <=1 semaphore wait per instruction (_split_excess_waits post-pass).
  - no matmul with rhs free size 1 (scalar reductions go through [1, 256]
    row-sums + a free-axis reduce; scalar broadcasts use [1, 2] operands).
  - no tensor_tensor_reduce (mask-mult + tensor_reduce / activation instead).
"""
import sys
import numpy as np

sys.path.insert(0, "/opt/trn_rl_repo")

import concourse.bass as bass
import concourse.mybir as mybir
import concourse.tile as tile
import bass_rust
from concourse.bass_utils import run_bass_kernel_spmd

dt = mybir.dt

N_CORES = 8
B, C, H, W_SP = 64, 256, 14, 14
M = H * W_SP           # 196
E = 512
K = C * C              # 65536
BL = B // N_CORES      # 8 samples per core
KL = K // N_CORES      # 8192 contraction slice per core
BN_EPS = 1e-5

_cache = {}


def _split_excess_waits(nc, max_waits=1):
    """walrus in this env rejects >1 semaphore wait per instruction; hoist
    excess waits onto preceding NoOps on the same engine."""
    for fn in nc.m.functions:
        for bb in fn.blocks:
            new_insts = []
            for inst in bb.instructions:
                si = inst.sync_info
                if si is not None and si.on_wait and len(si.on_wait) > max_waits:
                    waits = list(si.on_wait)
                    chunks = [waits[i:i + max_waits]
                              for i in range(0, len(waits), max_waits)]
                    for chunk in chunks[:-1]:
                        nop = mybir.InstNoOp(
                            name=nc.get_next_instruction_name(), ins=[], outs=[],
                            engine=inst.engine)
                        nop.sync_info = bass_rust.SyncInfo(on_wait=chunk,
                                                           on_update=[])
                        new_insts.append(nop)
                    si.on_wait = chunks[-1]
                new_insts.append(inst)
            bb.instructions = new_insts


def _build(stage=5):
    """stage: 1=Y0 dump, 2=F dump, 3=a2a_out dump, 4=emb partial dump,
    5=full kernel.

    All 256x256 matrices use a stacked-pair layout: S[p, r*256 + j] =
    X[128*r + p, j] -- one [128, 512] tile per matrix, so every elementwise
    op is a single instruction and every PSUM product fills one full bank."""
    f32, f32r = dt.float32, dt.float32r
    nc = bass.Bass("TRN2", target_bir_lowering=False, debug=False,
                   num_devices=N_CORES)

    featT = nc.dram_tensor("featT", [BL, M, C], f32r, kind="ExternalInput")
    onesc = nc.dram_tensor("onesc", [128, 1], f32r, kind="ExternalInput")
    onesr = nc.dram_tensor("onesr", [1, 128], f32r, kind="ExternalInput")
    ident3 = nc.dram_tensor("threeIS", [128, 2 * C], f32, kind="ExternalInput")
    if stage >= 4:
        wT = nc.dram_tensor("wT", [KL, E], f32r, kind="ExternalInput")
    if stage >= 5:
        bnsc = nc.dram_tensor("bnsc", [BL, E], f32, kind="ExternalInput")
        bnsh = nc.dram_tensor("bnsh", [BL, E], f32, kind="ExternalInput")
        out = nc.dram_tensor("out", [BL, E], f32, kind="ExternalOutput")
    elif stage <= 2:
        out = nc.dram_tensor("dbg", [2, 128, C], f32, kind="ExternalOutput")
    elif stage == 3:
        out = nc.dram_tensor("dbg", [128, 4096], f32, kind="ExternalOutput")
    else:
        out = nc.dram_tensor("dbg", [64, E], f32, kind="ExternalOutput")

    rg = [list(range(N_CORES))]
    AluOp = mybir.AluOpType
    NCH = KL // 128        # 64 k-chunks for the projection

    lp = nc.allow_low_precision(reason="f32r intermediates carry fp32 bits")
    lp.__enter__()
    with tile.TileContext(nc) as tc:
        with (
            tc.tile_pool(name="consts", bufs=1) as cpool,
            tc.tile_pool(name="wbuf", bufs=1) as wpool,
            tc.tile_pool(name="big", bufs=1) as bigpool,
            tc.tile_pool(name="work", bufs=3) as pool,
            tc.tile_pool(name="mats", bufs=2) as mats,
            tc.tile_pool(name="pss", bufs=2, space="PSUM") as pss,
            tc.tile_pool(name="psg", bufs=5, space="PSUM") as psg,
            tc.tile_pool(name="dram", bufs=1, space="DRAM") as dram,
        ):
            # ---------- constants ----------
            ones_t = cpool.tile([128, 1], f32r, name="ones_t")
            nc.sync.dma_start(ones_t[:], onesc[:])
            onesr_t = cpool.tile([1, 128], f32r, name="onesr_t")
            nc.sync.dma_start(onesr_t[:], onesr[:])
            threeIS_t = cpool.tile([128, 2 * C], f32, name="threeIS_t")
            nc.sync.dma_start(threeIS_t[:], ident3[:])
            if stage >= 5:
                bnsc_t = cpool.tile([BL, E], f32, name="bnsc_t")
                bnsh_t = cpool.tile([BL, E], f32, name="bnsh_t")
                nc.sync.dma_start(bnsc_t[:], bnsc[:])
                nc.sync.dma_start(bnsh_t[:], bnsh[:])

            # ---------- DRAM staging for collectives ----------
            if stage >= 3:
                # two half-batch exchanges: half q carries samples 4q..4q+3
                # flat layout per half: [j(8), h(2), p(128), b_l(4), i0(32)]
                a2a_in = [dram.tile([128, 2048], f32r, name=f"a2a_in{q}")
                          for q in range(2)]
                a2a_out = [dram.tile([128, 2048], f32r, name=f"a2a_out{q}")
                           for q in range(2)]
                a2a_in_v = [a2a_in[q].flatten().rearrange(
                    "(j h p b i) -> h b p j i", j=8, h=2, p=128, b=BL // 2,
                    i=32) for q in range(2)]
            if stage >= 5:
                rs_in = dram.tile([B, E], f32, name="rs_in")
                rs_out = dram.tile([BL, E], f32, name="rs_out")

            def mmp(outS, AS, BS):
                """outS = A @ B for symmetric A, all in stacked-pair layout."""
                for r in range(2):
                    for kc in range(2):
                        nc.tensor.matmul(
                            outS[:, C * r:C * (r + 1)],
                            AS[:, C * kc + 128 * r:C * kc + 128 * r + 128],
                            BS[:, C * kc:C * (kc + 1)],
                            start=(kc == 0), stop=(kc == 1))

            def scalar_bcast(val_sb, tag):
                """[1,1] f32r scalar -> [128,1] f32 SBUF (via N=2 matmul)."""
                v2 = pool.tile([1, 2], f32r, name=f"v2{tag}", tag=f"v2{tag}")
                nc.vector.tensor_copy(v2[:, 0:1], val_sb[:])
                nc.vector.tensor_copy(v2[:, 1:2], val_sb[:])
                b_ps = pss.tile([128, 2], f32, name=f"bps{tag}", tag="sm2", bufs=1)
                nc.tensor.matmul(b_ps[:], onesr_t[:], v2[:],
                                 start=True, stop=True)
                b_sb = pool.tile([128, 1], f32, name=f"bsb{tag}", tag=f"bsb{tag}")
                nc.vector.tensor_copy(b_sb[:], b_ps[:, 0:1])
                return b_sb

            # ---------- pooling phase: BL samples, stage-major in groups ----------
            nsamp = 1 if stage <= 2 else BL
            GD = min(4, nsamp)     # software-pipeline depth

            # preload every sample's feat tiles first (small DMAs ahead of
            # everything else in the queues)
            Bts = []
            for b in range(nsamp):
                B0 = pool.tile([128, C], f32r, name=f"B0_{b}", tag="B0",
                               bufs=nsamp)
                B1 = pool.tile([M - 128, C], f32r, name=f"B1_{b}", tag="B1",
                               bufs=nsamp)
                nc.sync.dma_start(B0[:], featT[b, 0:128, :])
                nc.sync.dma_start(B1[:], featT[b, 128:M, :])
                Bts.append((B0, B1))

            for g0 in range(0, nsamp, GD):
                gb = list(range(g0, min(g0 + GD, nsamp)))
                st = {b: {} for b in gb}

                # -- column sums
                for b in gb:
                    B0, B1 = Bts[b]
                    srow_ps = pss.tile([1, 2 * C], f32, name=f"srow{b}",
                                       tag="sm1")
                    nc.tensor.matmul(srow_ps[:, 0:C], ones_t[0:128, :], B0[:],
                                     start=True, stop=False)
                    nc.tensor.matmul(srow_ps[:, 0:C], ones_t[0:M - 128, :],
                                     B1[:], start=False, stop=True)
                    st[b]["srow"] = srow_ps
                for b in gb:
                    s_sb = pool.tile([1, C], f32r, name=f"s_sb{b}", tag="s_sb",
                                     bufs=GD)
                    t_sb = pool.tile([1, C], f32r, name=f"t_sb{b}", tag="t_sb",
                                     bufs=GD)
                    nc.scalar.copy(s_sb[:], st[b]["srow"][:, 0:C])
                    nc.scalar.mul(t_sb[:], st[b]["srow"][:, 0:C], -1.0 / M)
                    st[b]["s"], st[b]["t"] = s_sb, t_sb
                    # squares for the trace (parallel to G):
                    # tr(G) = ||A||_F^2 - ||srow||^2 / M
                    sqB0 = pool.tile([128, C], f32r, name=f"sqB0_{b}",
                                     tag="sqB0", bufs=GD)
                    sqB1 = pool.tile([M - 128, C], f32r, name=f"sqB1_{b}",
                                     tag="sqB1", bufs=GD)
                    s2 = pool.tile([1, C], f32, name=f"s2_{b}", tag="s2",
                                   bufs=GD)
                    B0, B1 = Bts[b]
                    nc.scalar.activation(sqB0[:], B0[:],
                                         mybir.ActivationFunctionType.Square)
                    nc.scalar.activation(sqB1[:], B1[:],
                                         mybir.ActivationFunctionType.Square)
                    nc.scalar.activation(s2[:], st[b]["srow"][:, 0:C],
                                         mybir.ActivationFunctionType.Square)
                    st[b]["sqB0"], st[b]["sqB1"], st[b]["s2"] = sqB0, sqB1, s2
                for b in gb:
                    trrow_ps = pss.tile([1, 2 * C], f32, name=f"trrow{b}",
                                        tag="sm1")
                    nc.tensor.matmul(trrow_ps[:, 0:C], ones_t[0:128, :],
                                     st[b]["sqB0"][:], start=True, stop=False)
                    nc.tensor.matmul(trrow_ps[:, 0:C], ones_t[0:M - 128, :],
                                     st[b]["sqB1"][:], start=False, stop=True)
                    st[b]["trrow"] = trrow_ps
                for b in gb:
                    asum = pool.tile([1, 1], f32, name=f"asum{b}", tag="asum",
                                     bufs=GD)
                    nc.vector.tensor_reduce(out=asum[:],
                                            in_=st[b]["trrow"][:, 0:C],
                                            axis=mybir.AxisListType.X,
                                            op=AluOp.add)
                    s2sum = pool.tile([1, 1], f32, name=f"s2sum{b}", tag="s2s",
                                      bufs=GD)
                    nc.vector.tensor_reduce(out=s2sum[:], in_=st[b]["s2"][:],
                                            axis=mybir.AxisListType.X,
                                            op=AluOp.add)
                    tr_sb = pool.tile([1, 1], f32, name=f"tr_sb{b}",
                                      tag="tr_sb", bufs=GD)
                    nc.vector.scalar_tensor_tensor(
                        out=tr_sb[:], in0=s2sum[:], scalar=-1.0 / M,
                        in1=asum[:], op0=AluOp.mult, op1=AluOp.add)
                    inv_sb = pool.tile([1, 1], f32r, name=f"inv_sb{b}",
                                       tag="inv", bufs=GD)
                    nc.vector.reciprocal(inv_sb[:], tr_sb[:])
                    st[b]["invb"] = scalar_bcast(inv_sb, f"i{b}")

                # -- G = A^T A - M xbar xbar^T
                for b in gb:
                    B0, B1 = Bts[b]
                    GS = psg.tile([128, 2 * C], f32, name=f"GS{b}", tag="Yp")
                    for r in range(2):
                        nc.tensor.matmul(GS[:, C * r:C * (r + 1)],
                                         B0[:, 128 * r:128 * (r + 1)], B0[:],
                                         start=True, stop=False)
                        nc.tensor.matmul(GS[:, C * r:C * (r + 1)],
                                         B1[:, 128 * r:128 * (r + 1)], B1[:],
                                         start=False, stop=False)
                        nc.tensor.matmul(GS[:, C * r:C * (r + 1)],
                                         st[b]["t"][:, 128 * r:128 * (r + 1)],
                                         st[b]["s"][:], start=False, stop=True)
                    st[b]["G"] = GS

                # -- Y0 = G/trG ; T1 = 3I - Y0
                for b in gb:
                    Y0S = mats.tile([128, 2 * C], f32r, name=f"Y0S{b}",
                                    tag="Y0", bufs=GD)
                    nc.vector.tensor_scalar_mul(Y0S[:], st[b]["G"][:],
                                                st[b]["invb"][:])
                    st[b]["Y0"] = Y0S

                if stage == 1:
                    for r in range(2):
                        nc.sync.dma_start(
                            out[r, :, :],
                            st[gb[0]]["Y0"][:, C * r:C * (r + 1)].bitcast(f32))
                    break

                for b in gb:
                    T1S = mats.tile([128, 2 * C], f32r, name=f"T1S{b}",
                                    tag="T1", bufs=GD)
                    nc.vector.scalar_tensor_tensor(
                        out=T1S[:], in0=st[b]["Y0"][:], scalar=-1.0,
                        in1=threeIS_t[:], op0=AluOp.mult, op1=AluOp.add)
                    st[b]["T1"] = T1S

                # -- deferred-scale NS: materialize unscaled products and
                # fold the 0.5 factors into the 3I-minus-scaled-product ops.
                #   UY1 = Y0 T1            (Y1 = .5 UY1)
                #   T2  = 3I - .25 T1 UY1  (= 3I - Z1 Y1)
                #   UY2 = UY1 T2           (Y2 = .25 UY2)
                #   UZ2 = T2 T1            (Z2 = .25 UZ2)
                #   T3  = 3I - 1/16 UZ2 UY2
                #   Y3 ~ UY2 T3            (global scale irrelevant)
                for b in gb:
                    YpS = psg.tile([128, 2 * C], f32, name=f"YpS{b}", tag="Yp")
                    mmp(YpS, st[b]["Y0"], st[b]["T1"])
                    st[b]["Yp"] = YpS
                for b in gb:
                    Y1S = mats.tile([128, 2 * C], f32r, name=f"Y1S{b}",
                                    tag="Y1", bufs=GD)
                    nc.scalar.copy(Y1S[:], st[b]["Yp"][:])
                    st[b]["Y1"] = Y1S

                # -- iter2
                for b in gb:
                    PpS = psg.tile([128, 2 * C], f32, name=f"PpS{b}", tag="Yp")
                    mmp(PpS, st[b]["T1"], st[b]["Y1"])
                    st[b]["Pp"] = PpS
                for b in gb:
                    T2S = mats.tile([128, 2 * C], f32r, name=f"T2S{b}",
                                    tag="T", bufs=GD)
                    nc.vector.scalar_tensor_tensor(
                        out=T2S[:], in0=st[b]["Pp"][:], scalar=-0.25,
                        in1=threeIS_t[:], op0=AluOp.mult, op1=AluOp.add)
                    st[b]["T2"] = T2S
                for b in gb:
                    Yp2S = psg.tile([128, 2 * C], f32, name=f"Yp2S{b}",
                                    tag="Yp")
                    mmp(Yp2S, st[b]["Y1"], st[b]["T2"])
                    st[b]["Yp2"] = Yp2S
                for b in gb:
                    Y2S = mats.tile([128, 2 * C], f32r, name=f"Y2S{b}",
                                    tag="Y2", bufs=GD)
                    nc.scalar.copy(Y2S[:], st[b]["Yp2"][:])
                    st[b]["Y2"] = Y2S
                for b in gb:
                    ZpS = psg.tile([128, 2 * C], f32, name=f"ZpS{b}", tag="Yp")
                    mmp(ZpS, st[b]["T2"], st[b]["T1"])
                    st[b]["Zp"] = ZpS
                for b in gb:
                    Z2S = mats.tile([128, 2 * C], f32r, name=f"Z2S{b}",
                                    tag="Z", bufs=GD)
                    nc.scalar.copy(Z2S[:], st[b]["Zp"][:])
                    st[b]["Z2"] = Z2S

                # -- iter3 (Z dead)
                for b in gb:
                    Pp3S = psg.tile([128, 2 * C], f32, name=f"Pp3S{b}",
                                    tag="Yp")
                    mmp(Pp3S, st[b]["Z2"], st[b]["Y2"])
                    st[b]["Pp3"] = Pp3S
                for b in gb:
                    T3S = mats.tile([128, 2 * C], f32r, name=f"T3S{b}",
                                    tag="T", bufs=GD)
                    nc.vector.scalar_tensor_tensor(
                        out=T3S[:], in0=st[b]["Pp3"][:], scalar=-1.0 / 16.0,
                        in1=threeIS_t[:], op0=AluOp.mult, op1=AluOp.add)
                    st[b]["T3"] = T3S
                for b in gb:
                    Y3pS = psg.tile([128, 2 * C], f32, name=f"Y3pS{b}",
                                    tag="Yp")
                    mmp(Y3pS, st[b]["Y2"], st[b]["T3"])
                    st[b]["Y3p"] = Y3pS

                # -- flat-normalize + staging
                for b in gb:
                    sqS = pool.tile([128, 2 * C], f32r, name=f"sqS{b}",
                                    tag="scr", bufs=GD)
                    nc.scalar.activation(sqS[:], st[b]["Y3p"][:],
                                         mybir.ActivationFunctionType.Square)
                    st[b]["sq"] = sqS
                for b in gb:
                    ssqrow_ps = pss.tile([1, 2 * C], f32, name=f"ssqrow{b}",
                                         tag="sm1")
                    nc.tensor.matmul(ssqrow_ps[:], ones_t[0:128, :],
                                     st[b]["sq"][:], start=True, stop=True)
                    st[b]["ssqrow"] = ssqrow_ps
                for b in gb:
                    ssq_sb = pool.tile([1, 1], f32, name=f"ssq_sb{b}",
                                       tag="tr_sb", bufs=GD)
                    nc.vector.tensor_reduce(out=ssq_sb[:],
                                            in_=st[b]["ssqrow"][:],
                                            axis=mybir.AxisListType.X,
                                            op=AluOp.add)
                    sqr_sb = pool.tile([1, 1], f32, name=f"sqr_sb{b}",
                                       tag="sqr", bufs=GD)
                    nc.scalar.sqrt(sqr_sb[:], ssq_sb[:])
                    rsq_sb = pool.tile([1, 1], f32r, name=f"rsq_sb{b}",
                                       tag="inv", bufs=GD)
                    nc.vector.reciprocal(rsq_sb[:], sqr_sb[:])
                    st[b]["rsqb"] = scalar_bcast(rsq_sb, f"r{b}")
                for b in gb:
                    FS = mats.tile([128, 2 * C], f32r, name=f"FS{b}", tag="F",
                                   bufs=GD)
                    nc.vector.tensor_scalar_mul(FS[:], st[b]["Y3p"][:],
                                                st[b]["rsqb"][:])
                    st[b]["F"] = FS

                if stage == 2:
                    for r in range(2):
                        nc.sync.dma_start(
                            out[r, :, :],
                            st[gb[0]]["F"][:, C * r:C * (r + 1)].bitcast(f32))
                    break

                for b in gb:
                    for hh in range(2):
                        nc.sync.dma_start(
                            a2a_in_v[b // (BL // 2)][hh, b % (BL // 2)],
                            st[b]["F"][:, C * hh:C * (hh + 1)]
                                .rearrange("p (j i) -> p j i", j=8, i=32))

            # ---------- AllToAll (two halves; first overlaps pooling) ----
            if stage >= 3:
                for q in range(2):
                    nc.gpsimd.collective_compute(
                        "AllToAll", AluOp.bypass, replica_groups=rg,
                        ins=[a2a_in[q].opt()], outs=[a2a_out[q].opt()])

            if stage == 3:
                tmp = bigpool.tile([128, 4096], f32, name="tmp")
                for q in range(2):
                    nc.sync.dma_start(tmp[:, 2048 * q:2048 * (q + 1)],
                                      a2a_out[q][:].bitcast(f32))
                nc.sync.dma_start(out[:], tmp[:])

            if stage >= 4:
                # ------- consumer: BIG [128, 4096], free = [h, s, b, i] -------
                BIG = bigpool.tile([128, 2 * 8 * BL * 32], f32r, name="BIG")
                a2a_out_v = [a2a_out[q].flatten().rearrange(
                    "(s h p b i) -> h s p b i", s=8, h=2, p=128, b=BL // 2,
                    i=32) for q in range(2)]
                BIG_v = BIG[:].rearrange("p (h s q b i) -> q h s p b i",
                                         h=2, s=8, q=2, b=BL // 2, i=32)
                for q in range(2):
                    for hh in range(2):
                        for s in range(8):
                            nc.sync.dma_start(BIG_v[q, hh, s],
                                              a2a_out_v[q][hh, s])

                # ------- projection: EMB[64, 512], W streamed -------
                EMB = pss.tile([64, E], f32, name="EMB", tag="sm1")
                BIG_k = BIG[:].rearrange("p (h sb i) -> h i p sb",
                                         h=2, sb=64, i=32)
                wT_v = wT.rearrange("(c p) e -> c p e", p=128)  # [64,128,512]
                wqs = []
                for c in range(NCH):
                    wq = wpool.tile([128, E], f32r, name=f"wq{c}", tag="wq",
                                    bufs=36)
                    nc.sync.dma_start(wq[:], wT_v[c])
                    wqs.append(wq)

                # PE warm-keepers: dep-free 512-row matmuls that fill the
                # AllToAll hole so the HAM clock gate stays open for the
                # projection matmuls that follow.
                warm_ps = pss.tile([1, E], f32, name="warm", tag="sm1")
                for wi in range(64):
                    nc.tensor.matmul(warm_ps[:], ones_t[0:128, :], wqs[0][:],
                                     start=True, stop=True)

                for c in range(NCH):
                    i_local, hh = c // 2, c % 2
                    nc.tensor.matmul(
                        EMB[:], BIG_k[hh, i_local], wqs[c][:],
                        start=(c == 0), stop=(c == NCH - 1))

                emb_sb = pool.tile([64, E], f32, name="emb_sb", tag="emb", bufs=1)
                nc.vector.tensor_copy(emb_sb[:], EMB[:])
                if stage == 4:
                    nc.sync.dma_start(out[:], emb_sb[:])

            if stage >= 5:
                nc.sync.dma_start(rs_in[:], emb_sb[:])

                # ------- ReduceScatter: [64, E] -> [8, E] -------
                nc.gpsimd.collective_compute(
                    "ReduceScatter", AluOp.add, replica_groups=rg,
                    ins=[rs_in.opt()], outs=[rs_out.opt()])

                # ------- BN fold + final L2 normalize -------
                e_sb = pool.tile([BL, E], f32, name="e_sb", tag="fin", bufs=1)
                nc.sync.dma_start(e_sb[:], rs_out[:])
                e_bn = pool.tile([BL, E], f32, name="e_bn", tag="fin2", bufs=1)
                nc.vector.tensor_tensor(e_bn[:], e_sb[:], bnsc_t[:], AluOp.mult)
                nc.vector.tensor_tensor(e_bn[:], e_bn[:], bnsh_t[:], AluOp.add)
                scr3 = pool.tile([BL, E], f32, name="scr3", tag="fin", bufs=1)
                nrm_sb = pool.tile([BL, 1], f32, name="nrm_sb", tag="nrm")
                nc.scalar.activation(
                    scr3[:], e_bn[:], mybir.ActivationFunctionType.Square,
                    accum_out=nrm_sb[:])
                nrms_sb = pool.tile([BL, 1], f32, name="nrms_sb", tag="nrms")
                nc.scalar.sqrt(nrms_sb[:], nrm_sb[:])
                rs_sb = pool.tile([BL, 1], f32, name="rs_sb", tag="nrmr")
                nc.vector.reciprocal(rs_sb[:], nrms_sb[:])
                e_fin = pool.tile([BL, E], f32, name="e_fin", tag="fin3", bufs=1)
                nc.vector.tensor_scalar_mul(e_fin[:], e_bn[:], rs_sb[:])
                nc.sync.dma_start(out[:], e_fin[:])

    _split_excess_waits(nc)
    return nc


def host_inputs(feat, W_proj, b_proj, bn_gamma, bn_beta, bn_mean, bn_var):
    """Build the 8 per-core input maps."""
    feat = np.ascontiguousarray(np.asarray(feat, dtype=np.float32))
    W_proj = np.asarray(W_proj, dtype=np.float32)
    featT = feat.reshape(B, C, M).transpose(0, 2, 1)          # [64, 196, 256]
    bnscale = (np.asarray(bn_gamma) /
               np.sqrt(np.asarray(bn_var) + BN_EPS)).astype(np.float32)
    bnshift = ((np.asarray(b_proj) - np.asarray(bn_mean)) * bnscale
               + np.asarray(bn_beta)).astype(np.float32)
    bnsc_rep = np.ascontiguousarray(np.broadcast_to(bnscale, (BL, E)))
    bnsh_rep = np.ascontiguousarray(np.broadcast_to(bnshift, (BL, E)))

    onesc = np.ones((128, 1), np.float32)
    onesr = np.ones((1, 128), np.float32)
    threeIS = np.zeros((128, 2 * C), np.float32)
    threeIS[:, 0:128] = 3.0 * np.eye(128, dtype=np.float32)
    threeIS[:, C + 128:C + 256] = 3.0 * np.eye(128, dtype=np.float32)

    in_maps = []
    for i in range(N_CORES):
        in_maps.append({
            "featT": np.ascontiguousarray(featT[i * BL:(i + 1) * BL]),
            "wT": np.ascontiguousarray(W_proj[:, KL * i:KL * (i + 1)].T),
            "onesc": onesc, "onesr": onesr, "threeIS": threeIS,
            "bnsc": bnsc_rep, "bnsh": bnsh_rep,
        })
    return in_maps


def kernel(feat, W_proj, b_proj, bn_gamma, bn_beta, bn_mean, bn_var):
    if "nc" not in _cache:
        _cache["nc"] = _build()
    nc = _cache["nc"]
    in_maps = host_inputs(feat, W_proj, b_proj, bn_gamma, bn_beta,
                          bn_mean, bn_var)
    last_err = None
    for _attempt in range(4):
        try:
            res = run_bass_kernel_spmd(nc, in_maps,
                                       core_ids=list(range(N_CORES)))
            break
        except Exception as e:  # transient NRT_EXEC_UNIT_UNRECOVERABLE flakes
            last_err = e
            import time as _time
            _time.sleep(2.0)
    else:
        raise last_err
    return np.concatenate([res.results[i]["out"] for i in range(N_CORES)],
                          axis=0)



# revision 14
# speedup vs baseline: 1.4565x; 1.4565x over previous
"""MPN-COV pooling + projection kernel for 8 Trainium2 NeuronCores.

Problem: nn_PillTeacher_48661979464182
  feat [64, 256, 14, 14] -> per-sample covariance + 3 Newton-Schulz sqrt
  iterations -> L2-normalize -> project with W_proj [512, 65536] -> BN -> L2.

Sharding:
  - Pooling: pure data parallel, 8 samples per core (two groups of 4).
  - Projection: k-shard of W_proj (each core holds an 8192-wide slice of the
    contraction dim, bf16). AllToAll per group exchanges the normalized
    pooled matrices (bf16); per-half ReduceScatter sums the partial
    embeddings back to the owning core.

v2 changes vs the first working kernel:
  - W / A2A payload / projection in bf16 (half the HBM + wire bytes).
  - W DMAs issued at kernel start on the sync queue (before they were stuck
    behind collective-gated staging DMAs and streamed during the projection).
  - Per-sample scalar chains (trace + F-norm reductions) batched across the
    group: accum_out activations + matmul partition-reductions + one
    broadcast matmul for all 4 samples.
  - A2A(q) triggered right after group q; BIG staging on the gpsimd queue;
    projection for half-batch 0 runs during A2A(1); keeper matmuls bridge
    PE idle gaps so the HAM clock gate stays open.
  - Two ReduceScatters (one per half-batch) in bf16; BN scale folded into W
    on the host.

Workarounds kept from v1:
  - <=1 semaphore wait per instruction (_split_excess_waits post-pass).
  - no matmul with rhs free size 1; scalar broadcasts via small matmuls.
"""
import sys
import numpy as np

sys.path.insert(0, "/opt/trn_rl_repo")

import concourse.bass as bass
import concourse.mybir as mybir
import concourse.tile as tile
import bass_rust
from concourse.bass_utils import run_bass_kernel_spmd

dt = mybir.dt

N_CORES = 8
B, C, H, W_SP = 64, 256, 14, 14
M = H * W_SP           # 196
E = 512
K = C * C              # 65536
BL = B // N_CORES      # 8 samples per core
KL = K // N_CORES      # 8192 contraction slice per core
GD = 4                 # group size (samples per A2A half)
BN_EPS = 1e-5

_cache = {}


def _split_excess_waits(nc, max_waits=1):
    """walrus in this env rejects >1 semaphore wait per instruction; hoist
    excess waits onto preceding NoOps on the same engine."""
    for fn in nc.m.functions:
        for bb in fn.blocks:
            new_insts = []
            for inst in bb.instructions:
                si = inst.sync_info
                if si is not None and si.on_wait and len(si.on_wait) > max_waits:
                    waits = list(si.on_wait)
                    chunks = [waits[i:i + max_waits]
                              for i in range(0, len(waits), max_waits)]
                    for chunk in chunks[:-1]:
                        nop = mybir.InstNoOp(
                            name=nc.get_next_instruction_name(), ins=[], outs=[],
                            engine=inst.engine)
                        nop.sync_info = bass_rust.SyncInfo(on_wait=chunk,
                                                           on_update=[])
                        new_insts.append(nop)
                    si.on_wait = chunks[-1]
                new_insts.append(inst)
            bb.instructions = new_insts


def _build():
    f32, f32r, bf16 = dt.float32, dt.float32r, dt.bfloat16
    nc = bass.Bass("TRN2", target_bir_lowering=False, debug=False,
                   num_devices=N_CORES)

    featT = nc.dram_tensor("featT", [BL, M, C], f32r, kind="ExternalInput")
    wT = nc.dram_tensor("wT", [KL, E], bf16, kind="ExternalInput")
    onesc = nc.dram_tensor("onesc", [128, 1], f32r, kind="ExternalInput")
    onesr = nc.dram_tensor("onesr", [1, 128], f32r, kind="ExternalInput")
    ident3 = nc.dram_tensor("threeIS", [128, 2 * C], f32, kind="ExternalInput")
    bnsh = nc.dram_tensor("bnsh", [GD, E], bf16, kind="ExternalInput")
    out = nc.dram_tensor("out", [BL, E], f32, kind="ExternalOutput")

    rg = [list(range(N_CORES))]
    AluOp = mybir.AluOpType
    Act = mybir.ActivationFunctionType
    NCH = KL // 128        # 64 k-chunks for the projection

    lp = nc.allow_low_precision(reason="f32r/bf16 intermediates")
    lp.__enter__()
    with tile.TileContext(nc) as tc:
        with (
            tc.tile_pool(name="consts", bufs=1) as cpool,
            tc.tile_pool(name="wbuf", bufs=1) as wpool,
            tc.tile_pool(name="big", bufs=1) as bigpool,
            tc.tile_pool(name="work", bufs=2) as pool,
            tc.tile_pool(name="mats", bufs=2) as mats,
            tc.tile_pool(name="psP", bufs=4, space="PSUM") as psg,
            tc.tile_pool(name="psS", bufs=1, space="PSUM") as pss,
            tc.tile_pool(name="psE", bufs=1, space="PSUM") as pse,
            tc.tile_pool(name="dram", bufs=1, space="DRAM") as dram,
        ):
            # ---------- constants + feat + W prefetch (sync queue) --------
            ones_t = cpool.tile([128, 1], f32r, name="ones_t")
            nc.sync.dma_start(ones_t[:], onesc[:])
            onesr_t = cpool.tile([1, 128], f32r, name="onesr_t")
            nc.sync.dma_start(onesr_t[:], onesr[:])
            threeIS_t = cpool.tile([128, 2 * C], f32, name="threeIS_t")
            nc.sync.dma_start(threeIS_t[:], ident3[:])
            bnsh_t = cpool.tile([GD, E], bf16, name="bnsh_t")
            nc.sync.dma_start(bnsh_t[:], bnsh[:])

            Bts = []
            for b in range(BL):
                B0 = pool.tile([128, C], f32r, name=f"B0_{b}", tag="B0",
                               bufs=BL)
                B1 = pool.tile([M - 128, C], f32r, name=f"B1_{b}", tag="B1",
                               bufs=BL)
                nc.sync.dma_start(B0[:], featT[b, 0:128, :])
                nc.sync.dma_start(B1[:], featT[b, 128:M, :])
                Bts.append((B0, B1))

            wT_v = wT.rearrange("(c p) e -> c p e", p=128)  # [64,128,512]
            wqs = []
            for c in range(NCH):
                wq = wpool.tile([128, E], bf16, name=f"wq{c}", tag="wq",
                                bufs=NCH)
                nc.sync.dma_start(wq[:], wT_v[c])
                wqs.append(wq)

            # ---------- DRAM staging for collectives ----------
            # a2a half q carries local samples 4q..4q+3.
            # flat layout per half: [j(8), h(2), p(128), b_l(4), i0(32)]
            a2a_in = [dram.tile([128, 2048], bf16, name=f"a2a_in{q}")
                      for q in range(2)]
            a2a_out = [dram.tile([128, 2048], bf16, name=f"a2a_out{q}")
                       for q in range(2)]
            a2a_in_v = [a2a_in[q].flatten().rearrange(
                "(j h p b i) -> h b p j i", j=8, h=2, p=128, b=GD,
                i=32) for q in range(2)]
            a2a_out_v = [a2a_out[q].flatten().rearrange(
                "(s h p b i) -> h s p b i", s=8, h=2, p=128, b=GD,
                i=32) for q in range(2)]
            rs_in = [dram.tile([32, E], bf16, name=f"rs_in{q}")
                     for q in range(2)]
            rs_out = [dram.tile([GD, E], bf16, name=f"rs_out{q}")
                      for q in range(2)]

            # BIG: projection lhsT source. cols = [q(2), hh(2), s(8), b(4), i(32)]
            # (s,b,i) contiguous per (q,hh) so the staging DMA collapses to
            # [128, 128] contiguous blocks; the lhsT slice has 2 free dims.
            BIG = bigpool.tile([128, 2 * 2 * 32 * 8 * GD], bf16, name="BIG")
            BIG_v = BIG[:].rearrange("p (q h s b i) -> q h s p b i",
                                     q=2, h=2, i=32, s=8, b=GD)

            # PSUM smalls: one bank, disjoint column regions.
            # cols 4:8 asum(g), 8:12 nrmsum(g) per group at offset 16*g;
            # cols 32+8g:+4 invb(g), 36+8g:+4 rsqb(g); cols 64:512 keepers.
            smalls = pss.tile([128, 512], f32, name="smalls")

            def mmp(outS, AS, BS):
                """outS = A @ B for symmetric A, stacked-pair layout."""
                for r in range(2):
                    for kc in range(2):
                        nc.tensor.matmul(
                            outS[:, C * r:C * (r + 1)],
                            AS[:, C * kc + 128 * r:C * kc + 128 * r + 128],
                            BS[:, C * kc:C * (kc + 1)],
                            start=(kc == 0), stop=(kc == 1))

            # =============== pooling phase: 2 groups of 4 ===============
            for g in range(2):
                gb = list(range(g * GD, (g + 1) * GD))
                st = {b: {} for b in gb}
                so = 16 * g   # smalls column offset for this group

                # -- per-sample column sums + s/t row copies
                s2a_row = pool.tile([1, GD], f32, name=f"s2a{g}", tag="s2a")
                q0_all = pool.tile([128, GD], f32r, name=f"q0a{g}", tag="q0a")
                q1_all = pool.tile([M - 128, GD], f32r, name=f"q1a{g}",
                                   tag="q1a")
                for bi, b in enumerate(gb):
                    B0, B1 = Bts[b]
                    srow_ps = pss.tile([1, 2 * C], f32, name=f"srow{b}",
                                       tag="srow", bufs=2)
                    nc.tensor.matmul(srow_ps[:, 0:C], ones_t[0:128, :], B0[:],
                                     start=True, stop=False)
                    nc.tensor.matmul(srow_ps[:, 0:C], ones_t[0:M - 128, :],
                                     B1[:], start=False, stop=True)
                    s_sb = pool.tile([1, C], f32r, name=f"s_sb{b}", tag="s_sb",
                                     bufs=GD)
                    t_sb = pool.tile([1, C], f32r, name=f"t_sb{b}", tag="t_sb",
                                     bufs=GD)
                    nc.scalar.copy(s_sb[:], srow_ps[:, 0:C])
                    nc.scalar.mul(t_sb[:], srow_ps[:, 0:C], -1.0 / M)
                    st[b]["s"], st[b]["t"] = s_sb, t_sb
                    scrS = pool.tile([1, C], bf16, name=f"scrS{b}", tag="scrS",
                                     bufs=2)
                    nc.scalar.activation(scrS[:], s_sb[:].bitcast(f32),
                                         Act.Square,
                                         accum_out=s2a_row[0:1, bi:bi + 1])
                    scr0 = pool.tile([128, C], bf16, name=f"scr0_{b}",
                                     tag="scr0", bufs=2)
                    scr1 = pool.tile([M - 128, C], bf16, name=f"scr1_{b}",
                                     tag="scr1", bufs=2)
                    nc.scalar.activation(
                        scr0[:], B0[:], Act.Square,
                        accum_out=q0_all[:, bi:bi + 1])
                    nc.scalar.activation(
                        scr1[:], B1[:], Act.Square,
                        accum_out=q1_all[:, bi:bi + 1])
                # asum[1,4] = sum_p q0 + sum_p q1
                nc.tensor.matmul(smalls[0:1, so + 4:so + 4 + GD],
                                 ones_t[0:128, :], q0_all[:],
                                 start=True, stop=False)
                nc.tensor.matmul(smalls[0:1, so + 4:so + 4 + GD],
                                 ones_t[0:M - 128, :], q1_all[:],
                                 start=False, stop=True)
                # tr = asum - s2/M;  inv = 1/tr
                tr_all = pool.tile([1, GD], f32, name=f"tr{g}", tag="tr")
                nc.vector.scalar_tensor_tensor(
                    out=tr_all[:], in0=s2a_row[:], scalar=-1.0 / M,
                    in1=smalls[0:1, so + 4:so + 4 + GD],
                    op0=AluOp.mult, op1=AluOp.add)
                inv_all = pool.tile([1, GD], f32r, name=f"inv{g}", tag="inv")
                nc.vector.reciprocal(inv_all[:], tr_all[:])
                nc.tensor.matmul(smalls[:, 32 + 8 * g:32 + 8 * g + GD],
                                 onesr_t[:], inv_all[:],
                                 start=True, stop=True)
                invb = pool.tile([128, GD], f32, name=f"invb{g}", tag="invb")
                nc.scalar.copy(invb[:], smalls[:, 32 + 8 * g:32 + 8 * g + GD])

                # -- G = A^T A - M xbar xbar^T  (per sample)
                for bi, b in enumerate(gb):
                    B0, B1 = Bts[b]
                    GS = psg.tile([128, 2 * C], f32, name=f"GS{b}", tag="Yp")
                    for r in range(2):
                        nc.tensor.matmul(GS[:, C * r:C * (r + 1)],
                                         B0[:, 128 * r:128 * (r + 1)], B0[:],
                                         start=True, stop=False)
                        nc.tensor.matmul(GS[:, C * r:C * (r + 1)],
                                         B1[:, 128 * r:128 * (r + 1)], B1[:],
                                         start=False, stop=False)
                        nc.tensor.matmul(
                            GS[:, C * r:C * (r + 1)],
                            st[b]["t"][:, 128 * r:128 * (r + 1)],
                            st[b]["s"][:], start=False, stop=True)
                    st[b]["G"] = GS

                # -- Y0 = G/trG ; T1 = 3I - Y0  (interleaved per sample)
                for bi, b in enumerate(gb):
                    Y0S = mats.tile([128, 2 * C], f32r, name=f"Y0S{b}",
                                    tag="Y0", bufs=GD)
                    nc.vector.tensor_scalar_mul(Y0S[:], st[b]["G"][:],
                                                invb[:, bi:bi + 1])
                    T1S = mats.tile([128, 2 * C], f32r, name=f"T1S{b}",
                                    tag="T1", bufs=GD)
                    nc.vector.scalar_tensor_tensor(
                        out=T1S[:], in0=Y0S[:], scalar=-1.0,
                        in1=threeIS_t[:], op0=AluOp.mult, op1=AluOp.add)
                    st[b]["Y0"], st[b]["T1"] = Y0S, T1S

                # -- deferred-scale NS:
                #   UY1 = Y0 T1;  T2 = 3I - .25 T1 UY1;  UY2 = UY1 T2
                #   UZ2 = T2 T1;  T3 = 3I - 1/16 UZ2 UY2;  Y3 ~ UY2 T3
                for b in gb:
                    YpS = psg.tile([128, 2 * C], f32, name=f"YpS{b}", tag="Yp")
                    mmp(YpS, st[b]["Y0"], st[b]["T1"])
                    st[b]["Yp"] = YpS
                for b in gb:
                    Y1S = mats.tile([128, 2 * C], f32r, name=f"Y1S{b}",
                                    tag="Y1", bufs=GD)
                    nc.scalar.copy(Y1S[:], st[b]["Yp"][:])
                    st[b]["Y1"] = Y1S
                for b in gb:
                    PpS = psg.tile([128, 2 * C], f32, name=f"PpS{b}", tag="Yp")
                    mmp(PpS, st[b]["T1"], st[b]["Y1"])
                    st[b]["Pp"] = PpS
                for b in gb:
                    T2S = mats.tile([128, 2 * C], f32r, name=f"T2S{b}",
                                    tag="T2", bufs=GD)
                    nc.vector.scalar_tensor_tensor(
                        out=T2S[:], in0=st[b]["Pp"][:], scalar=-0.25,
                        in1=threeIS_t[:], op0=AluOp.mult, op1=AluOp.add)
                    st[b]["T2"] = T2S
                for b in gb:
                    Yp2S = psg.tile([128, 2 * C], f32, name=f"Yp2S{b}",
                                    tag="Yp")
                    mmp(Yp2S, st[b]["Y1"], st[b]["T2"])
                    st[b]["Yp2"] = Yp2S
                for b in gb:
                    Y2S = mats.tile([128, 2 * C], f32r, name=f"Y2S{b}",
                                    tag="Y2", bufs=GD)
                    nc.scalar.copy(Y2S[:], st[b]["Yp2"][:])
                    st[b]["Y2"] = Y2S
                for b in gb:
                    ZpS = psg.tile([128, 2 * C], f32, name=f"ZpS{b}", tag="Yp")
                    mmp(ZpS, st[b]["T2"], st[b]["T1"])
                    st[b]["Zp"] = ZpS
                for b in gb:
                    Z2S = mats.tile([128, 2 * C], f32r, name=f"Z2S{b}",
                                    tag="Z2", bufs=GD)
                    nc.vector.tensor_copy(Z2S[:], st[b]["Zp"][:])
                    st[b]["Z2"] = Z2S
                for b in gb:
                    Pp3S = psg.tile([128, 2 * C], f32, name=f"Pp3S{b}",
                                    tag="Yp")
                    mmp(Pp3S, st[b]["Z2"], st[b]["Y2"])
                    st[b]["Pp3"] = Pp3S
                for b in gb:
                    T3S = mats.tile([128, 2 * C], f32r, name=f"T3S{b}",
                                    tag="T2", bufs=GD)
                    nc.vector.scalar_tensor_tensor(
                        out=T3S[:], in0=st[b]["Pp3"][:], scalar=-1.0 / 16.0,
                        in1=threeIS_t[:], op0=AluOp.mult, op1=AluOp.add)
                    st[b]["T3"] = T3S
                for b in gb:
                    Y3pS = psg.tile([128, 2 * C], f32, name=f"Y3pS{b}",
                                    tag="Yp")
                    mmp(Y3pS, st[b]["Y2"], st[b]["T3"])
                    st[b]["Y3p"] = Y3pS

                # -- batched flat-normalize
                nrm_all = pool.tile([128, GD], f32r, name=f"nrm{g}", tag="nrm")
                for bi, b in enumerate(gb):
                    scrq = pool.tile([128, 2 * C], bf16, name=f"scrq{b}",
                                     tag="scrq", bufs=2)
                    nc.scalar.activation(
                        scrq[:], st[b]["Y3p"][:], Act.Square,
                        accum_out=nrm_all[:, bi:bi + 1])
                nc.tensor.matmul(smalls[0:1, so + 8:so + 8 + GD],
                                 ones_t[0:128, :], nrm_all[:],
                                 start=True, stop=True)
                sq_sb = pool.tile([1, GD], f32, name=f"sq{g}", tag="sq")
                nc.scalar.sqrt(sq_sb[:], smalls[0:1, so + 8:so + 8 + GD])
                rsq_all = pool.tile([1, GD], f32r, name=f"rsq{g}", tag="rsq")
                nc.vector.reciprocal(rsq_all[:], sq_sb[:])
                nc.tensor.matmul(smalls[:, 36 + 8 * g:36 + 8 * g + GD],
                                 onesr_t[:], rsq_all[:],
                                 start=True, stop=True)
                rsqb = pool.tile([128, GD], f32, name=f"rsqb{g}", tag="rsqb")
                nc.scalar.copy(rsqb[:], smalls[:, 36 + 8 * g:36 + 8 * g + GD])

                for bi, b in enumerate(gb):
                    FS = mats.tile([128, 2 * C], bf16, name=f"FS{b}", tag="F",
                                   bufs=GD)
                    nc.vector.tensor_scalar_mul(FS[:], st[b]["Y3p"][:],
                                                rsqb[:, bi:bi + 1])
                    for hh in range(2):
                        nc.sync.dma_start(
                            a2a_in_v[g][hh, bi],
                            FS[:, C * hh:C * (hh + 1)]
                                .rearrange("p (j i) -> p j i", j=8, i=32))

                # -- AllToAll for this half + BIG staging (gpsimd queue)
                nc.gpsimd.collective_compute(
                    "AllToAll", AluOp.bypass, replica_groups=rg,
                    ins=[a2a_in[g].opt()], outs=[a2a_out[g].opt()])
                for hh in range(2):
                    for s in range(8):
                        nc.gpsimd.dma_start(BIG_v[g, hh, s],
                                            a2a_out_v[g][hh, s])

            # =============== projection ===============
            # keepers: dep-free f32 matmuls bridging the PE gap between the
            # end of pooling and the BIG0-gated projection start.
            for _ in range(4):
                nc.tensor.matmul(smalls[0:1, 64:512], threeIS_t[:, 0:1],
                                 threeIS_t[:, 0:448], start=True, stop=True)

            BIG_k = BIG[:].rearrange("p (q h s b i) -> q h i p s b",
                                     q=2, h=2, s=8, b=GD, i=32)
            emb_sb = []
            for q in range(2):
                EMBq = pse.tile([32, E], f32, name=f"EMB{q}", tag="EMB")
                for c in range(NCH):
                    nc.tensor.matmul(
                        EMBq[:], BIG_k[q, c % 2, c // 2], wqs[c][:],
                        start=(c == 0), stop=(c == NCH - 1))
                eq = pool.tile([32, E], bf16, name=f"emb{q}", tag="emb",
                               bufs=2)
                nc.vector.tensor_copy(eq[:], EMBq[:])
                nc.scalar.dma_start(rs_in[q][:], eq[:])
                emb_sb.append(eq)
                if q == 0:
                    # keepers to bridge EMB0 -> BIG1-gated EMB1
                    for _ in range(3):
                        nc.tensor.matmul(smalls[0:1, 64:512],
                                         threeIS_t[:, 0:1],
                                         threeIS_t[:, 0:448],
                                         start=True, stop=True)

            # =============== ReduceScatter + finalize (per half) ==========
            for q in range(2):
                nc.gpsimd.collective_compute(
                    "ReduceScatter", AluOp.add, replica_groups=rg,
                    ins=[rs_in[q].opt()], outs=[rs_out[q].opt()])
            for q in range(2):
                e_sb = pool.tile([GD, E], bf16, name=f"e_sb{q}", tag="fin",
                                 bufs=2)
                nc.scalar.dma_start(e_sb[:], rs_out[q][:])
                e_bn = pool.tile([GD, E], f32, name=f"e_bn{q}", tag="fin2",
                                 bufs=2)
                nc.vector.tensor_tensor(e_bn[:], e_sb[:], bnsh_t[:],
                                        AluOp.add)
                scr3 = pool.tile([GD, E], bf16, name=f"scr3{q}", tag="fin3",
                                 bufs=2)
                nrm2 = pool.tile([GD, 1], f32, name=f"nrm2{q}", tag="nrm2",
                                 bufs=2)
                nc.scalar.activation(scr3[:], e_bn[:], Act.Square,
                                     accum_out=nrm2[:])
                nrm2s = pool.tile([GD, 1], f32, name=f"nrm2s{q}", tag="nrm2s",
                                  bufs=2)
                nc.scalar.sqrt(nrm2s[:], nrm2[:])
                rsf = pool.tile([GD, 1], f32, name=f"rsf{q}", tag="rsf",
                                bufs=2)
                nc.vector.reciprocal(rsf[:], nrm2s[:])
                e_fin = pool.tile([GD, E], f32, name=f"e_fin{q}", tag="fin4",
                                  bufs=2)
                nc.vector.tensor_scalar_mul(e_fin[:], e_bn[:], rsf[:])
                nc.scalar.dma_start(out[GD * q:GD * (q + 1), :], e_fin[:])

    _split_excess_waits(nc)
    return nc


def host_inputs(feat, W_proj, b_proj, bn_gamma, bn_beta, bn_mean, bn_var):
    """Build the 8 per-core input maps."""
    import ml_dtypes
    bf16 = ml_dtypes.bfloat16
    feat = np.ascontiguousarray(np.asarray(feat, dtype=np.float32))
    W_proj = np.asarray(W_proj, dtype=np.float32)
    featT = feat.reshape(B, C, M).transpose(0, 2, 1)          # [64, 196, 256]
    bnscale = (np.asarray(bn_gamma) /
               np.sqrt(np.asarray(bn_var) + BN_EPS)).astype(np.float32)
    bnshift = ((np.asarray(b_proj) - np.asarray(bn_mean)) * bnscale
               + np.asarray(bn_beta)).astype(np.float32)
    bnsh_rep = np.ascontiguousarray(
        np.broadcast_to(bnshift, (GD, E))).astype(bf16)
    W_scaled = W_proj * bnscale[:, None]                      # fold BN scale

    onesc = np.ones((128, 1), np.float32)
    onesr = np.ones((1, 128), np.float32)
    threeIS = np.zeros((128, 2 * C), np.float32)
    threeIS[:, 0:128] = 3.0 * np.eye(128, dtype=np.float32)
    threeIS[:, C + 128:C + 256] = 3.0 * np.eye(128, dtype=np.float32)

    in_maps = []
    for i in range(N_CORES):
        in_maps.append({
            "featT": np.ascontiguousarray(featT[i * BL:(i + 1) * BL]),
            "wT": np.ascontiguousarray(
                W_scaled[:, KL * i:KL * (i + 1)].T).astype(bf16),
            "onesc": onesc, "onesr": onesr, "threeIS": threeIS,
            "bnsh": bnsh_rep,
        })
    return in_maps


def kernel(feat, W_proj, b_proj, bn_gamma, bn_beta, bn_mean, bn_var):
    if "nc" not in _cache:
        _cache["nc"] = _build()
    nc = _cache["nc"]
    in_maps = host_inputs(feat, W_proj, b_proj, bn_gamma, bn_beta,
                          bn_mean, bn_var)
    last_err = None
    for _attempt in range(4):
        try:
            res = run_bass_kernel_spmd(nc, in_maps,
                                       core_ids=list(range(N_CORES)))
            break
        except Exception as e:  # transient NRT_EXEC_UNIT_UNRECOVERABLE flakes
            last_err = e
            import time as _time
            _time.sleep(2.0)
    else:
        raise last_err
    return np.concatenate([res.results[i]["out"] for i in range(N_CORES)],
                          axis=0)


# revision 15
# speedup vs baseline: 1.7000x; 1.1672x over previous
"""MPN-COV pooling + projection kernel for 8 Trainium2 NeuronCores.

Problem: nn_PillTeacher_48661979464182
  feat [64, 256, 14, 14] -> per-sample covariance + 3 Newton-Schulz sqrt
  iterations -> L2-normalize -> project with W_proj [512, 65536] -> BN -> L2.

Sharding:
  - Pooling: pure data parallel, 8 samples per core (two groups of 4).
  - Projection: k-shard of W_proj (each core holds an 8192-wide slice of the
    contraction dim, bf16). AllToAll per group exchanges the normalized
    pooled matrices (bf16); per-half ReduceScatter sums the partial
    embeddings back to the owning core.

v2 changes vs the first working kernel:
  - W / A2A payload / projection in bf16 (half the HBM + wire bytes).
  - W DMAs issued at kernel start on the sync queue (before they were stuck
    behind collective-gated staging DMAs and streamed during the projection).
  - Per-sample scalar chains (trace + F-norm reductions) batched across the
    group: accum_out activations + matmul partition-reductions + one
    broadcast matmul for all 4 samples.
  - A2A(q) triggered right after group q; BIG staging on the gpsimd queue;
    projection for half-batch 0 runs during A2A(1); keeper matmuls bridge
    PE idle gaps so the HAM clock gate stays open.
  - Two ReduceScatters (one per half-batch) in bf16; BN scale folded into W
    on the host.

Workarounds kept from v1:
  - <=1 semaphore wait per instruction (_split_excess_waits post-pass).
  - no matmul with rhs free size 1; scalar broadcasts via small matmuls.
"""
import sys
import numpy as np

sys.path.insert(0, "/opt/trn_rl_repo")

import concourse.bass as bass
import concourse.mybir as mybir
import concourse.tile as tile
import bass_rust
from concourse.bass_utils import run_bass_kernel_spmd

dt = mybir.dt

N_CORES = 8
B, C, H, W_SP = 64, 256, 14, 14
M = H * W_SP           # 196
E = 512
K = C * C              # 65536
BL = B // N_CORES      # 8 samples per core
KL = K // N_CORES      # 8192 contraction slice per core
GD = 4                 # group size (samples per A2A half)
BN_EPS = 1e-5

_cache = {}


def _split_excess_waits(nc, max_waits=1):
    """walrus in this env rejects >1 semaphore wait per instruction; hoist
    excess waits onto preceding NoOps on the same engine."""
    for fn in nc.m.functions:
        for bb in fn.blocks:
            new_insts = []
            for inst in bb.instructions:
                si = inst.sync_info
                if si is not None and si.on_wait and len(si.on_wait) > max_waits:
                    waits = list(si.on_wait)
                    chunks = [waits[i:i + max_waits]
                              for i in range(0, len(waits), max_waits)]
                    for chunk in chunks[:-1]:
                        nop = mybir.InstNoOp(
                            name=nc.get_next_instruction_name(), ins=[], outs=[],
                            engine=inst.engine)
                        nop.sync_info = bass_rust.SyncInfo(on_wait=chunk,
                                                           on_update=[])
                        new_insts.append(nop)
                    si.on_wait = chunks[-1]
                new_insts.append(inst)
            bb.instructions = new_insts


def _build():
    f32, f32r, bf16 = dt.float32, dt.float32r, dt.bfloat16
    nc = bass.Bass("TRN2", target_bir_lowering=False, debug=False,
                   num_devices=N_CORES)

    featT = nc.dram_tensor("featT", [BL, M, C], f32r, kind="ExternalInput")
    wT = nc.dram_tensor("wT", [KL, E], bf16, kind="ExternalInput")
    onesc = nc.dram_tensor("onesc", [128, 1], f32r, kind="ExternalInput")
    onesr = nc.dram_tensor("onesr", [1, 128], f32r, kind="ExternalInput")
    ident3 = nc.dram_tensor("threeIS", [128, 2 * C], f32, kind="ExternalInput")
    bnsh = nc.dram_tensor("bnsh", [BL, E], bf16, kind="ExternalInput")
    out = nc.dram_tensor("out", [BL, E], f32, kind="ExternalOutput")

    rg = [list(range(N_CORES))]
    AluOp = mybir.AluOpType
    Act = mybir.ActivationFunctionType
    NCH = KL // 128        # 64 k-chunks for the projection

    lp = nc.allow_low_precision(reason="f32r/bf16 intermediates")
    lp.__enter__()
    with tile.TileContext(nc) as tc:
        with (
            tc.tile_pool(name="consts", bufs=1) as cpool,
            tc.tile_pool(name="wbuf", bufs=1) as wpool,
            tc.tile_pool(name="big", bufs=1) as bigpool,
            tc.tile_pool(name="work", bufs=2) as pool,
            tc.tile_pool(name="mats", bufs=2) as mats,
            tc.tile_pool(name="psP", bufs=4, space="PSUM") as psg,
            tc.tile_pool(name="psS", bufs=1, space="PSUM") as pss,
            tc.tile_pool(name="psE", bufs=1, space="PSUM") as pse,
            tc.tile_pool(name="dram", bufs=1, space="DRAM") as dram,
        ):
            # ---------- constants + feat + W prefetch (sync queue) --------
            ones_t = cpool.tile([128, 1], f32r, name="ones_t")
            nc.sync.dma_start(ones_t[:], onesc[:])
            onesr_t = cpool.tile([1, 128], f32r, name="onesr_t")
            nc.sync.dma_start(onesr_t[:], onesr[:])
            threeIS_t = cpool.tile([128, 2 * C], f32, name="threeIS_t")
            nc.sync.dma_start(threeIS_t[:], ident3[:])
            bnsh_t = cpool.tile([BL, E], bf16, name="bnsh_t")
            nc.sync.dma_start(bnsh_t[:], bnsh[:])

            Bts = []
            for b in range(BL):
                B0 = pool.tile([128, C], f32r, name=f"B0_{b}", tag="B0",
                               bufs=BL)
                B1 = pool.tile([M - 128, C], f32r, name=f"B1_{b}", tag="B1",
                               bufs=BL)
                nc.sync.dma_start(B0[:], featT[b, 0:128, :])
                nc.sync.dma_start(B1[:], featT[b, 128:M, :])
                Bts.append((B0, B1))

            wT_v = wT.rearrange("(c p) e -> c p e", p=128)  # [64,128,512]
            wqs = []
            for c in range(NCH):
                wq = wpool.tile([128, E], bf16, name=f"wq{c}", tag="wq",
                                bufs=NCH)
                nc.sync.dma_start(wq[:], wT_v[c])
                wqs.append(wq)

            # ---------- DRAM staging for collectives ----------
            # a2a half q carries local samples 4q..4q+3.
            # flat layout per half: [j(8), h(2), p(128), b_l(4), i0(32)]
            a2a_in = [dram.tile([128, 2048], bf16, name=f"a2a_in{q}")
                      for q in range(2)]
            a2a_out = [dram.tile([128, 2048], bf16, name=f"a2a_out{q}")
                       for q in range(2)]
            a2a_in_v = [a2a_in[q].flatten().rearrange(
                "(j h p b i) -> h b p j i", j=8, h=2, p=128, b=GD,
                i=32) for q in range(2)]
            a2a_out_v = [a2a_out[q].flatten().rearrange(
                "(s h p b i) -> h s p b i", s=8, h=2, p=128, b=GD,
                i=32) for q in range(2)]
            rs_in = dram.tile([B, E], bf16, name="rs_in")
            rs_out = dram.tile([BL, E], bf16, name="rs_out")
            # emb store target: half q covers global rows 8s+4q+b
            rs_in_v = rs_in.rearrange("(s h b) e -> h s b e", s=8, h=2,
                                      b=GD)

            # BIG: projection lhsT source. cols = [q(2), hh(2), s(8), b(4), i(32)]
            # (s,b,i) contiguous per (q,hh) so the staging DMA collapses to
            # [128, 128] contiguous blocks; the lhsT slice has 2 free dims.
            BIG = bigpool.tile([128, 2 * 2 * 32 * 8 * GD], bf16, name="BIG")

            # PSUM smalls: one bank, disjoint column regions.
            # cols 4:8 asum(g), 8:12 nrmsum(g) per group at offset 16*g;
            # cols 32+8g:+4 invb(g), 36+8g:+4 rsqb(g); cols 64:512 keepers.
            smalls = pss.tile([128, 512], f32, name="smalls")

            def mmp(outS, AS, BS):
                """outS = A @ B for symmetric A, stacked-pair layout."""
                for r in range(2):
                    for kc in range(2):
                        nc.tensor.matmul(
                            outS[:, C * r:C * (r + 1)],
                            AS[:, C * kc + 128 * r:C * kc + 128 * r + 128],
                            BS[:, C * kc:C * (kc + 1)],
                            start=(kc == 0), stop=(kc == 1))

            def keeper(n):
                """dep-free f32 matmuls (~0.75us each) that keep the PE HAM
                activity window from going idle across small gaps."""
                for _ in range(n):
                    nc.tensor.matmul(smalls[0:1, 64:512], threeIS_t[:, 0:1],
                                     threeIS_t[:, 0:448], start=True,
                                     stop=True)

            # =============== pooling phase: 2 groups of 4 ===============
            st = {b: {} for b in range(BL)}

            def emit_squares(g):
                """scalar: B-tile squares (dep on feat DMAs only)."""
                gq0 = pool.tile([128, GD], f32r, name=f"q0a{g}", tag="q0a")
                gq1 = pool.tile([M - 128, GD], f32r, name=f"q1a{g}",
                                tag="q1a")
                st[f"q0_{g}"], st[f"q1_{g}"] = gq0, gq1
                for bi, b in enumerate(range(g * GD, (g + 1) * GD)):
                    B0, B1 = Bts[b]
                    scr0 = pool.tile([128, C], bf16, name=f"scr0_{b}",
                                     tag="scr0", bufs=2)
                    scr1 = pool.tile([M - 128, C], bf16, name=f"scr1_{b}",
                                     tag="scr1", bufs=2)
                    nc.scalar.activation(scr0[:], B0[:], Act.Square,
                                         accum_out=gq0[:, bi:bi + 1])
                    nc.scalar.activation(scr1[:], B1[:], Act.Square,
                                         accum_out=gq1[:, bi:bi + 1])

            def emit_srow(g):
                """PE: per-sample column sums (dep on feat DMAs only)."""
                for b in range(g * GD, (g + 1) * GD):
                    B0, B1 = Bts[b]
                    srow_ps = pss.tile([1, 2 * C], f32, name=f"srow{b}",
                                       tag="srow", bufs=2)
                    nc.tensor.matmul(srow_ps[:, 0:C], ones_t[0:128, :],
                                     B0[:], start=True, stop=False)
                    nc.tensor.matmul(srow_ps[:, 0:C], ones_t[0:M - 128, :],
                                     B1[:], start=False, stop=True)
                    st[b]["srow"] = srow_ps

            def emit_scopies(g):
                """scalar: s copies + scrS; vector: t muls."""
                s2a_row = pool.tile([1, GD], f32, name=f"s2a{g}", tag="s2a")
                st[f"s2a_{g}"] = s2a_row
                for bi, b in enumerate(range(g * GD, (g + 1) * GD)):
                    srow_ps = st[b]["srow"]
                    s_sb = pool.tile([1, C], f32r, name=f"s_sb{b}",
                                     tag="s_sb", bufs=GD)
                    nc.scalar.copy(s_sb[:], srow_ps[:, 0:C])
                    scrS = pool.tile([1, C], bf16, name=f"scrS{b}",
                                     tag="scrS", bufs=2)
                    nc.scalar.activation(scrS[:], s_sb[:].bitcast(f32),
                                         Act.Square,
                                         accum_out=s2a_row[0:1, bi:bi + 1])
                    t_sb = pool.tile([1, C], f32r, name=f"t_sb{b}",
                                     tag="t_sb", bufs=GD)
                    nc.vector.tensor_scalar_mul(t_sb[:], srow_ps[:, 0:C],
                                                -1.0 / M)
                    st[b]["s"], st[b]["t"] = s_sb, t_sb

            def emit_mid(g):
                """trace chain + G + NS iterations + scrq squares."""
                gb = list(range(g * GD, (g + 1) * GD))
                so = 16 * g
                # asum[1,4] = sum_p q0 + sum_p q1
                nc.tensor.matmul(smalls[0:1, so + 4:so + 4 + GD],
                                 ones_t[0:128, :], st[f"q0_{g}"][:],
                                 start=True, stop=False)
                nc.tensor.matmul(smalls[0:1, so + 4:so + 4 + GD],
                                 ones_t[0:M - 128, :], st[f"q1_{g}"][:],
                                 start=False, stop=True)
                tr_all = pool.tile([1, GD], f32, name=f"tr{g}", tag="tr")
                nc.vector.scalar_tensor_tensor(
                    out=tr_all[:], in0=st[f"s2a_{g}"][:], scalar=-1.0 / M,
                    in1=smalls[0:1, so + 4:so + 4 + GD],
                    op0=AluOp.mult, op1=AluOp.add)
                inv_all = pool.tile([1, GD], f32r, name=f"inv{g}", tag="inv")
                nc.vector.reciprocal(inv_all[:], tr_all[:])
                nc.tensor.matmul(smalls[:, 32 + 8 * g:32 + 8 * g + GD],
                                 onesr_t[:], inv_all[:],
                                 start=True, stop=True)
                invb = pool.tile([128, GD], f32, name=f"invb{g}", tag="invb")
                nc.scalar.copy(invb[:],
                               smalls[:, 32 + 8 * g:32 + 8 * g + GD])

                # G = A^T A - M xbar xbar^T
                for bi, b in enumerate(gb):
                    B0, B1 = Bts[b]
                    GS = psg.tile([128, 2 * C], f32, name=f"GS{b}", tag="Yp")
                    for r in range(2):
                        nc.tensor.matmul(GS[:, C * r:C * (r + 1)],
                                         B0[:, 128 * r:128 * (r + 1)], B0[:],
                                         start=True, stop=False)
                        nc.tensor.matmul(GS[:, C * r:C * (r + 1)],
                                         B1[:, 128 * r:128 * (r + 1)], B1[:],
                                         start=False, stop=False)
                        nc.tensor.matmul(
                            GS[:, C * r:C * (r + 1)],
                            st[b]["t"][:, 128 * r:128 * (r + 1)],
                            st[b]["s"][:], start=False, stop=True)
                    st[b]["G"] = GS

                # Y0 = G/trG ; T1 = 3I - Y0
                for bi, b in enumerate(gb):
                    Y0S = mats.tile([128, 2 * C], f32r, name=f"Y0S{b}",
                                    tag="Y0", bufs=GD)
                    nc.vector.tensor_scalar_mul(Y0S[:], st[b]["G"][:],
                                                invb[:, bi:bi + 1])
                    T1S = mats.tile([128, 2 * C], f32r, name=f"T1S{b}",
                                    tag="T1", bufs=GD)
                    nc.vector.scalar_tensor_tensor(
                        out=T1S[:], in0=Y0S[:], scalar=-1.0,
                        in1=threeIS_t[:], op0=AluOp.mult, op1=AluOp.add)
                    st[b]["Y0"], st[b]["T1"] = Y0S, T1S

                # deferred-scale NS:
                #   UY1 = Y0 T1;  T2 = 3I - .25 T1 UY1;  UY2 = UY1 T2
                #   UZ2 = T2 T1;  T3 = 3I - 1/16 UZ2 UY2;  Y3 ~ UY2 T3
                for b in gb:
                    YpS = psg.tile([128, 2 * C], f32, name=f"YpS{b}",
                                   tag="Yp")
                    mmp(YpS, st[b]["Y0"], st[b]["T1"])
                    st[b]["Yp"] = YpS
                for b in gb:
                    Y1S = mats.tile([128, 2 * C], f32r, name=f"Y1S{b}",
                                    tag="Y1", bufs=GD)
                    nc.scalar.copy(Y1S[:], st[b]["Yp"][:])
                    st[b]["Y1"] = Y1S
                for b in gb:
                    PpS = psg.tile([128, 2 * C], f32, name=f"PpS{b}",
                                   tag="Yp")
                    mmp(PpS, st[b]["T1"], st[b]["Y1"])
                    st[b]["Pp"] = PpS
                for b in gb:
                    T2S = mats.tile([128, 2 * C], f32r, name=f"T2S{b}",
                                    tag="T2", bufs=GD)
                    nc.vector.scalar_tensor_tensor(
                        out=T2S[:], in0=st[b]["Pp"][:], scalar=-0.25,
                        in1=threeIS_t[:], op0=AluOp.mult, op1=AluOp.add)
                    st[b]["T2"] = T2S
                for b in gb:
                    Yp2S = psg.tile([128, 2 * C], f32, name=f"Yp2S{b}",
                                    tag="Yp")
                    mmp(Yp2S, st[b]["Y1"], st[b]["T2"])
                    st[b]["Yp2"] = Yp2S
                for b in gb:
                    Y2S = mats.tile([128, 2 * C], f32r, name=f"Y2S{b}",
                                    tag="Y2", bufs=GD)
                    nc.scalar.copy(Y2S[:], st[b]["Yp2"][:])
                    st[b]["Y2"] = Y2S
                for b in gb:
                    ZpS = psg.tile([128, 2 * C], f32, name=f"ZpS{b}",
                                   tag="Yp")
                    mmp(ZpS, st[b]["T2"], st[b]["T1"])
                    st[b]["Zp"] = ZpS
                for b in gb:
                    Z2S = mats.tile([128, 2 * C], f32r, name=f"Z2S{b}",
                                    tag="Z2", bufs=GD)
                    nc.vector.tensor_copy(Z2S[:], st[b]["Zp"][:])
                    st[b]["Z2"] = Z2S
                for b in gb:
                    Pp3S = psg.tile([128, 2 * C], f32, name=f"Pp3S{b}",
                                    tag="Yp")
                    mmp(Pp3S, st[b]["Z2"], st[b]["Y2"])
                    st[b]["Pp3"] = Pp3S
                for b in gb:
                    T3S = mats.tile([128, 2 * C], f32r, name=f"T3S{b}",
                                    tag="T2", bufs=GD)
                    nc.vector.scalar_tensor_tensor(
                        out=T3S[:], in0=st[b]["Pp3"][:], scalar=-1.0 / 16.0,
                        in1=threeIS_t[:], op0=AluOp.mult, op1=AluOp.add)
                    st[b]["T3"] = T3S
                for b in gb:
                    Y3pS = psg.tile([128, 2 * C], f32, name=f"Y3pS{b}",
                                    tag="Yp")
                    mmp(Y3pS, st[b]["Y2"], st[b]["T3"])
                    st[b]["Y3p"] = Y3pS
                # squared-sum accumulation for the flat L2 norm
                nrm_all = pool.tile([128, GD], f32r, name=f"nrm{g}",
                                    tag="nrm")
                st[f"nrm_{g}"] = nrm_all
                for bi, b in enumerate(gb):
                    scrq = pool.tile([128, 2 * C], bf16, name=f"scrq{b}",
                                     tag="scrq", bufs=2)
                    nc.scalar.activation(scrq[:], st[b]["Y3p"][:],
                                         Act.Square,
                                         accum_out=nrm_all[:, bi:bi + 1])

            def emit_tail(g):
                """flat-norm reduce + broadcast (PE/scalar/vector smalls)."""
                so = 16 * g
                nc.tensor.matmul(smalls[0:1, so + 8:so + 8 + GD],
                                 ones_t[0:128, :], st[f"nrm_{g}"][:],
                                 start=True, stop=True)
                sq_sb = pool.tile([1, GD], f32, name=f"sq{g}", tag="sq")
                nc.scalar.sqrt(sq_sb[:], smalls[0:1, so + 8:so + 8 + GD])
                rsq_all = pool.tile([1, GD], f32r, name=f"rsq{g}", tag="rsq")
                nc.vector.reciprocal(rsq_all[:], sq_sb[:])
                nc.tensor.matmul(smalls[:, 36 + 8 * g:36 + 8 * g + GD],
                                 onesr_t[:], rsq_all[:],
                                 start=True, stop=True)
                rsqb = pool.tile([128, GD], f32, name=f"rsqb{g}", tag="rsqb")
                nc.scalar.copy(rsqb[:],
                               smalls[:, 36 + 8 * g:36 + 8 * g + GD])
                st[f"rsqb_{g}"] = rsqb

            def emit_F(g):
                """vector: F = Y3p * rsqb (bf16) + exchange-layout DMAs."""
                rsqb = st[f"rsqb_{g}"]
                for bi, b in enumerate(range(g * GD, (g + 1) * GD)):
                    FS = mats.tile([128, 2 * C], bf16, name=f"FS{b}",
                                   tag="F", bufs=GD)
                    nc.vector.tensor_scalar_mul(FS[:], st[b]["Y3p"][:],
                                                rsqb[:, bi:bi + 1])
                    for hh in range(2):
                        nc.sync.dma_start(
                            a2a_in_v[g][hh, bi],
                            FS[:, C * hh:C * (hh + 1)]
                                .rearrange("p (j i) -> p j i", j=8, i=32))

            def emit_cc(g):
                """gpsimd: AllToAll(g) + BIG staging (2 bulk DMAs)."""
                nc.gpsimd.collective_compute(
                    "AllToAll", AluOp.bypass, replica_groups=rg,
                    ins=[a2a_in[g].opt()], outs=[a2a_out[g].opt()])
                stag_src = a2a_out[g].flatten().rearrange(
                    "(s h p c) -> h p s c", s=8, h=2, p=128, c=128)
                stag_dst = BIG[:, 2048 * g:2048 * (g + 1)].rearrange(
                    "p (h s c) -> h p s c", h=2, s=8, c=128)
                for hh in range(2):
                    nc.gpsimd.dma_start(stag_dst[hh], stag_src[hh])

            # -- interleaved emission: group g's tail overlaps group g+1's
            # head so the PE FIFO never head-of-line blocks long enough to
            # drop the HAM clock gate.
            emit_squares(0)
            emit_srow(0)
            emit_scopies(0)
            emit_mid(0)
            emit_srow(1)
            emit_tail(0)
            keeper(2)
            emit_F(0)
            emit_cc(0)
            emit_squares(1)
            emit_scopies(1)
            emit_mid(1)
            emit_tail(1)
            keeper(2)
            emit_F(1)
            emit_cc(1)
            keeper(2)

            # =============== projection ===============
            BIG_k = BIG[:].rearrange("p (q h s b i) -> q h i p s b",
                                     q=2, h=2, s=8, b=GD, i=32)
            for q in range(2):
                EMBq = pse.tile([32, E], f32, name=f"EMB{q}", tag="EMB")
                for c in range(NCH):
                    nc.tensor.matmul(
                        EMBq[:], BIG_k[q, c % 2, c // 2], wqs[c][:],
                        start=(c == 0), stop=(c == NCH - 1))
                eq = pool.tile([32, E], bf16, name=f"emb{q}", tag="emb",
                               bufs=2)
                nc.vector.tensor_copy(eq[:], EMBq[:])
                nc.scalar.dma_start(rs_in_v[q], eq[:])
                if q == 0:
                    keeper(3)

            # =============== ReduceScatter + finalize ==============
            nc.gpsimd.collective_compute(
                "ReduceScatter", AluOp.add, replica_groups=rg,
                ins=[rs_in.opt()], outs=[rs_out.opt()])
            e_sb = pool.tile([BL, E], bf16, name="e_sb", tag="fin")
            nc.scalar.dma_start(e_sb[:], rs_out[:])
            e_bn = pool.tile([BL, E], f32, name="e_bn", tag="fin2")
            nc.vector.tensor_tensor(e_bn[:], e_sb[:], bnsh_t[:], AluOp.add)
            scr3 = pool.tile([BL, E], bf16, name="scr3", tag="fin3")
            nrm2 = pool.tile([BL, 1], f32, name="nrm2", tag="nrm2")
            nc.scalar.activation(scr3[:], e_bn[:], Act.Square,
                                 accum_out=nrm2[:])
            nrm2s = pool.tile([BL, 1], f32, name="nrm2s", tag="nrm2s")
            nc.scalar.sqrt(nrm2s[:], nrm2[:])
            rsf = pool.tile([BL, 1], f32, name="rsf", tag="rsf")
            nc.vector.reciprocal(rsf[:], nrm2s[:])
            e_fin = pool.tile([BL, E], f32, name="e_fin", tag="fin4")
            nc.vector.tensor_scalar_mul(e_fin[:], e_bn[:], rsf[:])
            nc.scalar.dma_start(out[:], e_fin[:])

    _split_excess_waits(nc)
    return nc


def host_inputs(feat, W_proj, b_proj, bn_gamma, bn_beta, bn_mean, bn_var):
    """Build the 8 per-core input maps."""
    import ml_dtypes
    bf16 = ml_dtypes.bfloat16
    feat = np.ascontiguousarray(np.asarray(feat, dtype=np.float32))
    W_proj = np.asarray(W_proj, dtype=np.float32)
    featT = feat.reshape(B, C, M).transpose(0, 2, 1)          # [64, 196, 256]
    bnscale = (np.asarray(bn_gamma) /
               np.sqrt(np.asarray(bn_var) + BN_EPS)).astype(np.float32)
    bnshift = ((np.asarray(b_proj) - np.asarray(bn_mean)) * bnscale
               + np.asarray(bn_beta)).astype(np.float32)
    bnsh_rep = np.ascontiguousarray(
        np.broadcast_to(bnshift, (BL, E))).astype(bf16)
    W_scaled = W_proj * bnscale[:, None]                      # fold BN scale

    onesc = np.ones((128, 1), np.float32)
    onesr = np.ones((1, 128), np.float32)
    threeIS = np.zeros((128, 2 * C), np.float32)
    threeIS[:, 0:128] = 3.0 * np.eye(128, dtype=np.float32)
    threeIS[:, C + 128:C + 256] = 3.0 * np.eye(128, dtype=np.float32)

    in_maps = []
    for i in range(N_CORES):
        in_maps.append({
            "featT": np.ascontiguousarray(featT[i * BL:(i + 1) * BL]),
            "wT": np.ascontiguousarray(
                W_scaled[:, KL * i:KL * (i + 1)].T).astype(bf16),
            "onesc": onesc, "onesr": onesr, "threeIS": threeIS,
            "bnsh": bnsh_rep,
        })
    return in_maps


def kernel(feat, W_proj, b_proj, bn_gamma, bn_beta, bn_mean, bn_var):
    if "nc" not in _cache:
        _cache["nc"] = _build()
    nc = _cache["nc"]
    in_maps = host_inputs(feat, W_proj, b_proj, bn_gamma, bn_beta,
                          bn_mean, bn_var)
    last_err = None
    for _attempt in range(4):
        try:
            res = run_bass_kernel_spmd(nc, in_maps,
                                       core_ids=list(range(N_CORES)))
            break
        except Exception as e:  # transient NRT_EXEC_UNIT_UNRECOVERABLE flakes
            last_err = e
            import time as _time
            _time.sleep(2.0)
    else:
        raise last_err
    return np.concatenate([res.results[i]["out"] for i in range(N_CORES)],
                          axis=0)
